# revision 7
# baseline (speedup 1.0000x reference)
"""Contrastive-loss kernel for Trainium2 (8 NeuronCores, SPMD data-parallel).

Math (from the reference):
    diag_A_is = (A_is_t + A_is_t_14 + A_is_t_28)[i, i, :]        # [B, D]
    diag_A_em = (A_em_t + A_em_t_14 + A_em_t_28)[i, i, :]        # [B, D]
    loss = sum_b relu( sum_d (0.4*m + 0.6*tr_m) * (diag_A_is - diag_A_em) )

Only the diagonals A[i, i, :] of the six [B, B, D] tensors are touched
(1/256th of the data).  Sharding strategy: batch-dim data parallel across
the 8 cores — the host gathers the diagonal rows (pure data movement) and
ships each core its 32 rows of the eight [B, D] operands packed into a
single [128, 2080] fp32 tile; all arithmetic runs on-device.  Per-core
partial losses are summed on the host (8 scalars).

Factoring used on device:  0.4*m + 0.6*tr_m = 0.4 * (m + 1.5*tr_m), and
relu(0.4*x) = 0.4*relu(x), so the 0.4 is folded into the final fused
relu+scale+accumulate op.

Per-core device layout: X [128 partitions, 2080 fp32]
  cols [256*i, 256*(i+1)) for i in 0..7:  m, tr_m, is0, is1, is2, em0, em1, em2
      each a [32, 1024] row-block flattened row-major to [128, 256]
      (partition p = 4*row + quarter, 256 contiguous d's per partition)
  cols [2048, 2080): selector matrix E[p, b] = 1.0 iff p // 4 == b,
      used as matmul rhs to sum the four per-partition quarter-row dots
      of each batch row into one PSUM value (partition-group reduction).
"""

import numpy as np

import concourse.bass as bass
import concourse.mybir as mybir
from concourse.bass_utils import run_bass_kernel_spmd
from concourse.tile import TileContext

B = 256
D = 1024
N_CORES = 8
ROWS_PER_CORE = B // N_CORES  # 32
BLK = 256  # free-dim width of one packed [32, 1024] operand block
N_BLOCKS = 8  # m, tr_m, 3x A_is diag, 3x A_em diag
E_COLS = ROWS_PER_CORE  # 32
FREE = N_BLOCKS * BLK + E_COLS  # 2080

_NC_CACHE = None


def build_nc() -> bass.Bass:
    """Raw-bass SPMD kernel (no TileContext).

    Raw bass is used deliberately: this walrus build enforces a tiny
    per-instruction sync-wait limit (the fused f32 LDWEIGHTS takes 1 wait,
    the Tile kernel-tail Drain needs one wait per live semaphore and dies
    at 4), so Tile's auto-generated epilogue cannot compile here.  With
    explicit blocks every wait is its own instruction and the kernel needs
    only 3 semaphores.  It also avoids Tile's multi-microsecond tail
    barrier.  Same-engine DVE chains are safe without waits (each DVE op
    ends with a pipeline DRAIN before the next can issue).
    """
    f32 = mybir.dt.float32
    Alu = mybir.AluOpType

    nc = bass.Bass()
    x = nc.dram_tensor("x", [128, FREE], f32, kind="ExternalInput")
    out_d = nc.dram_tensor("out", [1, 1], f32, kind="ExternalOutput")

    with (
        nc.sbuf_tensor("xt", [128, FREE], f32) as xt,
        nc.sbuf_tensor("w", [128, BLK], f32) as w,
        nc.sbuf_tensor("diff", [128, 3, BLK], f32) as diff,
        nc.sbuf_tensor("prod", [128, 3, BLK], f32) as prod,
        nc.sbuf_tensor("rowq", [128, 1], f32) as rowq,
        nc.sbuf_tensor("srelu", [1, E_COLS], f32) as srelu,
        nc.sbuf_tensor("total", [1, 1], f32) as total,
        nc.psum_tensor("ps", [1, E_COLS], f32) as ps,
        nc.semaphore("dma_sem") as dma_sem,
        nc.semaphore("v_sem") as v_sem,
        nc.semaphore("pe_sem") as pe_sem,
        nc.Block() as block,
    ):
        # [128, 8, 256] view of the packed operand blocks
        xt3 = xt[:, : N_BLOCKS * BLK].rearrange("p (b f) -> p b f", f=BLK)
        w3 = w[:].rearrange("p (o f) -> p o f", o=1).broadcast_to([128, 3, BLK])

        @block.sync
        def _(sync):
            sync.dma_start(out=xt[:], in_=x[:]).then_inc(dma_sem, 16)
            sync.wait_ge(v_sem, 5)
            sync.dma_start(out=out_d[:], in_=total[:]).then_inc(dma_sem, 16)
            sync.wait_ge(dma_sem, 32)

        @block.vector
        def _(vector):
            vector.wait_ge(dma_sem, 16)
            # w = m + 1.5 * tr_m   (0.4 * w is deferred: relu(0.4 x) = 0.4 relu(x))
            nc.vector.scalar_tensor_tensor(
                out=w[:], in0=xt3[:, 1, :], scalar=1.5, in1=xt3[:, 0, :],
                op0=Alu.mult, op1=Alu.add,
            ).then_inc(v_sem, 1)
            # diff = (is0, is1, is2) - (em0, em1, em2)   one [128, 768] op
            nc.vector.tensor_sub(diff[:], xt3[:, 2:5, :], xt3[:, 5:8, :]).then_inc(
                v_sem, 1
            )
            vector.wait_ge(v_sem, 2)
            # prod = diff * w (w broadcast over the 3 sub-blocks)
            nc.vector.tensor_mul(prod[:], diff[:], w3).then_inc(v_sem, 1)
            vector.wait_ge(v_sem, 3)
            # rowq[p] = sum over the 768 = quarter-row dot product
            nc.vector.tensor_reduce(
                rowq[:], prod[:], axis=mybir.AxisListType.XY, op=Alu.add
            ).then_inc(v_sem, 1)
            vector.wait_ge(pe_sem, 1)
            # relu the 32 per-row sums, accumulate them to one scalar
            nc.vector.tensor_scalar(
                out=srelu[:], in0=ps[:], scalar1=0.0, scalar2=None,
                op0=Alu.max, op1=Alu.add, accum_out=total[:],
            ).then_inc(v_sem, 1)

        @block.tensor
        def _(tensor):
            tensor.wait_ge(dma_sem, 16)
            tensor.wait_ge(v_sem, 4)
            # s[1, 32] = rowq^T @ E — sums each row's 4 partition-quarters
            nc.tensor.matmul(
                ps[:], rowq[:], xt[:, N_BLOCKS * BLK :], start=True, stop=True
            ).then_inc(pe_sem, 1)

    return nc


def pack_inputs(A_is_t, A_is_t_14, A_is_t_28, A_em_t, A_em_t_14, A_em_t_28, m, tr_m):
    idx = np.arange(B)

    def diag(a):
        return np.asarray(a)[idx, idx]  # [B, D] gather of the used diagonal

    blocks = [
        np.asarray(m, dtype=np.float32),
        np.asarray(tr_m, dtype=np.float32),
        diag(A_is_t),
        diag(A_is_t_14),
        diag(A_is_t_28),
        diag(A_em_t),
        diag(A_em_t_14),
        diag(A_em_t_28),
    ]
    X = np.empty((N_CORES, 128, FREE), dtype=np.float32)
    for i, arr in enumerate(blocks):
        X[:, :, BLK * i : BLK * (i + 1)] = arr.reshape(N_CORES, 128, BLK)
    X[:, :, N_BLOCKS * BLK :] = np.repeat(np.eye(E_COLS, dtype=np.float32), 4, axis=0)
    return [{"x": np.ascontiguousarray(X[c])} for c in range(N_CORES)]


def run(in_maps, **kwargs):
    global _NC_CACHE
    if _NC_CACHE is None:
        _NC_CACHE = build_nc()
    return run_bass_kernel_spmd(
        _NC_CACHE, in_maps, core_ids=list(range(N_CORES)), **kwargs
    )


def kernel(**inputs) -> np.ndarray:
    res = run(pack_inputs(**inputs))
    total = 0.4 * sum(float(r["out"][0, 0]) for r in res.results)
    return np.array([total], dtype=np.float32)


# revision 13
# speedup vs baseline: 1.0490x; 1.0490x over previous
"""Contrastive-loss kernel for Trainium2 (8 NeuronCores, SPMD data-parallel).

Math (from the reference):
    diag_A_is = (A_is_t + A_is_t_14 + A_is_t_28)[i, i, :]        # [B, D]
    diag_A_em = (A_em_t + A_em_t_14 + A_em_t_28)[i, i, :]        # [B, D]
    loss = sum_b relu( sum_d (0.4*m + 0.6*tr_m) * (diag_A_is - diag_A_em) )

Only the diagonals A[i, i, :] of the six [B, B, D] tensors are touched
(1/256th of the data).  Sharding strategy: batch-dim data parallel across
the 8 cores — the host gathers the diagonal rows (pure data movement) and
ships each core its 32 rows of the eight [B, D] operands packed into a
single [128, 2080] fp32 tile; all arithmetic runs on-device.  Per-core
partial losses are summed on the host (8 scalars).

Factoring used on device:  0.4*m + 0.6*tr_m = 0.4 * (m + 1.5*tr_m), and
relu(0.4*x) = 0.4*relu(x), so the 0.4 is folded into the final fused
relu+scale+accumulate op.

Per-core device layout: X [128 partitions, 2080 fp32]
  cols [256*i, 256*(i+1)) for i in 0..7:  m, tr_m, is0, is1, is2, em0, em1, em2
      each a [32, 1024] row-block flattened row-major to [128, 256]
      (partition p = 4*row + quarter, 256 contiguous d's per partition)
  cols [2048, 2080): selector matrix E[p, b] = 1.0 iff p // 4 == b,
      used as matmul rhs to sum the four per-partition quarter-row dots
      of each batch row into one PSUM value (partition-group reduction).
"""

import numpy as np

import concourse.bass as bass
import concourse.mybir as mybir
from concourse.bass_utils import run_bass_kernel_spmd
from concourse.tile import TileContext

B = 256
D = 1024
N_CORES = 8
ROWS_PER_CORE = B // N_CORES  # 32
BLK = 256  # free-dim width of one packed [32, 1024] operand block
N_BLOCKS = 8  # m, tr_m, 3x A_is diag, 3x A_em diag
E_COLS = ROWS_PER_CORE  # 32
FREE = N_BLOCKS * BLK + E_COLS  # 2080

_NC_CACHE = None


def build_nc() -> bass.Bass:
    """Raw-bass SPMD kernel (no TileContext).

    Raw bass is used deliberately: this walrus build enforces a tiny
    per-instruction sync-wait limit (the fused f32 LDWEIGHTS takes 1 wait,
    the Tile kernel-tail Drain needs one wait per live semaphore and dies
    at 4), so Tile's auto-generated epilogue cannot compile here.  With
    explicit blocks every wait is its own instruction.  It also avoids
    Tile's multi-microsecond tail barrier.

    The 1.06 MB input load is split into 4 DMAs, two per HWDGE ring
    (sync + scalar), and the DVE work is pipelined per chunk so most of
    the arithmetic hides under the remaining transfers.
    """
    f32 = mybir.dt.float32
    Alu = mybir.AluOpType

    nc = bass.Bass()
    x = nc.dram_tensor("x", [128, FREE], f32, kind="ExternalInput")
    out_d = nc.dram_tensor("out", [1, 1], f32, kind="ExternalOutput")

    with (
        nc.sbuf_tensor("xt", [128, FREE], f32) as xt,
        nc.sbuf_tensor("w", [128, BLK], f32) as w,
        nc.sbuf_tensor("diff", [128, 3, BLK], f32) as diff,
        nc.sbuf_tensor("prod", [128, 3, BLK], f32) as prod,
        nc.sbuf_tensor("rowq", [128, 1], f32) as rowq,
        nc.sbuf_tensor("srelu", [1, E_COLS], f32) as srelu,
        nc.sbuf_tensor("total", [1, 1], f32) as total,
        nc.psum_tensor("ps", [1, E_COLS], f32) as ps,
        nc.semaphore("s1") as s1,  # sync ring: m/tr/E load (+out store later)
        nc.semaphore("s2") as s2,  # sync ring: is1/em1 load
        nc.semaphore("a1") as a1,  # scalar ring: is0/em0 load
        nc.semaphore("a2") as a2,  # scalar ring: is2/em2 load
        nc.semaphore("v_sem") as v_sem,
        nc.semaphore("pe_sem") as pe_sem,
        nc.Block() as block,
    ):
        # packed layout: m | tr | E | is0 em0 | is1 em1 | is2 em2
        m_ap = xt[:, 0:BLK]
        tr_ap = xt[:, BLK : 2 * BLK]
        e_ap = xt[:, 2 * BLK : 2 * BLK + E_COLS]
        o = 2 * BLK + E_COLS  # 544

        def pair(i):  # (is_i, em_i)
            a = o + 2 * BLK * i
            return xt[:, a : a + BLK], xt[:, a + BLK : a + 2 * BLK]

        w3 = w[:].rearrange("p (o f) -> p o f", o=1).broadcast_to([128, 3, BLK])

        @block.sync
        def _(sync):
            sync.dma_start(out=xt[:, 0:o], in_=x[:, 0:o]).then_inc(s1, 16)
            sync.dma_start(
                out=xt[:, o + 2 * BLK : o + 4 * BLK],
                in_=x[:, o + 2 * BLK : o + 4 * BLK],
            ).then_inc(s2, 16)
            sync.wait_ge(v_sem, 9)
            sync.dma_start(out=out_d[:], in_=total[:]).then_inc(s1, 16)
            sync.wait_ge(s1, 32)

        @block.scalar
        def _(scalar):
            scalar.dma_start(
                out=xt[:, o : o + 2 * BLK], in_=x[:, o : o + 2 * BLK]
            ).then_inc(a1, 16)
            scalar.dma_start(
                out=xt[:, o + 4 * BLK :], in_=x[:, o + 4 * BLK :]
            ).then_inc(a2, 16)

        @block.vector
        def _(vector):
            is0, em0 = pair(0)
            is1, em1 = pair(1)
            is2, em2 = pair(2)
            # w = m + 1.5 * tr_m   (0.4 is deferred: relu(0.4 x) = 0.4 relu(x))
            vector.wait_ge(s1, 16)
            nc.vector.scalar_tensor_tensor(
                out=w[:], in0=tr_ap, scalar=1.5, in1=m_ap,
                op0=Alu.mult, op1=Alu.add,
            ).then_inc(v_sem, 1)
            vector.wait_ge(a1, 16)
            nc.vector.tensor_sub(diff[:, 0, :], is0, em0).then_inc(v_sem, 1)
            vector.wait_ge(v_sem, 2)
            nc.vector.tensor_mul(prod[:, 0, :], diff[:, 0, :], w[:]).then_inc(v_sem, 1)
            vector.wait_ge(s2, 16)
            nc.vector.tensor_sub(diff[:, 1, :], is1, em1).then_inc(v_sem, 1)
            vector.wait_ge(v_sem, 4)
            nc.vector.tensor_mul(prod[:, 1, :], diff[:, 1, :], w[:]).then_inc(v_sem, 1)
            vector.wait_ge(a2, 16)
            nc.vector.tensor_sub(diff[:, 2, :], is2, em2).then_inc(v_sem, 1)
            vector.wait_ge(v_sem, 6)
            nc.vector.tensor_mul(prod[:, 2, :], diff[:, 2, :], w[:]).then_inc(v_sem, 1)
            vector.wait_ge(v_sem, 7)
            # rowq[p] = per-partition quarter-row dot product
            nc.vector.tensor_reduce(
                rowq[:], prod[:], axis=mybir.AxisListType.XY, op=Alu.add
            ).then_inc(v_sem, 1)
            vector.wait_ge(pe_sem, 1)
            # relu the 32 per-row sums, accumulate them to one scalar
            nc.vector.tensor_scalar(
                out=srelu[:], in0=ps[:], scalar1=0.0, scalar2=None,
                op0=Alu.max, op1=Alu.add, accum_out=total[:],
            ).then_inc(v_sem, 1)

        @block.tensor
        def _(tensor):
            tensor.wait_ge(s1, 16)
            tensor.wait_ge(v_sem, 8)
            # s[1, 32] = rowq^T @ E — sums each row's 4 partition-quarters
            nc.tensor.matmul(
                ps[:], rowq[:], e_ap, start=True, stop=True
            ).then_inc(pe_sem, 1)

    return nc


def pack_inputs(A_is_t, A_is_t_14, A_is_t_28, A_em_t, A_em_t_14, A_em_t_28, m, tr_m):
    idx = np.arange(B)

    def diag(a):
        return np.asarray(a)[idx, idx]  # [B, D] gather of the used diagonal

    # layout: m | tr | E | is0 em0 | is1 em1 | is2 em2
    blocks = [
        (0, np.asarray(m, dtype=np.float32)),
        (BLK, np.asarray(tr_m, dtype=np.float32)),
        (2 * BLK + E_COLS, diag(A_is_t)),
        (3 * BLK + E_COLS, diag(A_em_t)),
        (4 * BLK + E_COLS, diag(A_is_t_14)),
        (5 * BLK + E_COLS, diag(A_em_t_14)),
        (6 * BLK + E_COLS, diag(A_is_t_28)),
        (7 * BLK + E_COLS, diag(A_em_t_28)),
    ]
    X = np.empty((N_CORES, 128, FREE), dtype=np.float32)
    for off, arr in blocks:
        X[:, :, off : off + BLK] = arr.reshape(N_CORES, 128, BLK)
    X[:, :, 2 * BLK : 2 * BLK + E_COLS] = np.repeat(
        np.eye(E_COLS, dtype=np.float32), 4, axis=0
    )
    return [{"x": np.ascontiguousarray(X[c])} for c in range(N_CORES)]


def run(in_maps, **kwargs):
    global _NC_CACHE
    if _NC_CACHE is None:
        _NC_CACHE = build_nc()
    return run_bass_kernel_spmd(
        _NC_CACHE, in_maps, core_ids=list(range(N_CORES)), **kwargs
    )


def kernel(**inputs) -> np.ndarray:
    res = run(pack_inputs(**inputs))
    total = 0.4 * sum(float(r["out"][0, 0]) for r in res.results)
    return np.array([total], dtype=np.float32)


# revision 16
# speedup vs baseline: 1.0994x; 1.0481x over previous
"""Contrastive-loss kernel for Trainium2 (8 NeuronCores, SPMD data-parallel).

Math (from the reference):
    diag_A_is = (A_is_t + A_is_t_14 + A_is_t_28)[i, i, :]        # [B, D]
    diag_A_em = (A_em_t + A_em_t_14 + A_em_t_28)[i, i, :]        # [B, D]
    loss = sum_b relu( sum_d (0.4*m + 0.6*tr_m) * (diag_A_is - diag_A_em) )

Only the diagonals A[i, i, :] of the six [B, B, D] tensors are touched
(1/256th of the data).  Sharding strategy: batch-dim data parallel across
the 8 cores — the host gathers the diagonal rows (pure data movement) and
ships each core its 32 rows of the eight [B, D] operands packed into a
single [128, 2080] fp32 tile; all arithmetic runs on-device.  Per-core
partial losses are summed on the host (8 scalars).

Factoring used on device:  0.4*m + 0.6*tr_m = 0.4 * (m + 1.5*tr_m), and
relu(0.4*x) = 0.4*relu(x), so the 0.4 is folded into the final fused
relu+scale+accumulate op.

Per-core device layout: X [128 partitions, 2080 fp32]
  cols [256*i, 256*(i+1)) for i in 0..7:  m, tr_m, is0, is1, is2, em0, em1, em2
      each a [32, 1024] row-block flattened row-major to [128, 256]
      (partition p = 4*row + quarter, 256 contiguous d's per partition)
  cols [2048, 2080): selector matrix E[p, b] = 1.0 iff p // 4 == b,
      used as matmul rhs to sum the four per-partition quarter-row dots
      of each batch row into one PSUM value (partition-group reduction).
"""

import numpy as np

import concourse.bass as bass
import concourse.mybir as mybir
from concourse.bass_utils import run_bass_kernel_spmd
from concourse.tile import TileContext

B = 256
D = 1024
N_CORES = 8
ROWS_PER_CORE = B // N_CORES  # 32
BLK = 256  # free-dim width of one packed [32, 1024] operand block
N_BLOCKS = 8  # m, tr_m, 3x A_is diag, 3x A_em diag
E_COLS = ROWS_PER_CORE  # 32
FREE = N_BLOCKS * BLK + E_COLS  # 2080

_NC_CACHE = None


def build_nc() -> bass.Bass:
    """Raw-bass SPMD kernel (no TileContext).

    Raw bass is used deliberately: this walrus build enforces a tiny
    per-instruction sync-wait limit (the fused f32 LDWEIGHTS takes 1 wait,
    the Tile kernel-tail Drain needs one wait per live semaphore and dies
    at 4), so Tile's auto-generated epilogue cannot compile here.  With
    explicit blocks every wait is its own instruction.  It also avoids
    Tile's multi-microsecond tail barrier.

    The 1.06 MB input load is split into 4 DMAs, two per HWDGE ring
    (sync + scalar), and the DVE work is pipelined per chunk so most of
    the arithmetic hides under the remaining transfers.
    """
    f32 = mybir.dt.float32
    Alu = mybir.AluOpType

    nc = bass.Bass()
    x = nc.dram_tensor("x", [128, FREE], f32, kind="ExternalInput")
    out_d = nc.dram_tensor("out", [1, 1], f32, kind="ExternalOutput")

    with (
        nc.sbuf_tensor("xt", [128, FREE], f32) as xt,
        nc.sbuf_tensor("w", [128, BLK], f32) as w,
        nc.sbuf_tensor("diff", [128, 3, BLK], f32) as diff,
        nc.sbuf_tensor("prod", [128, 3, BLK], f32) as prod,
        nc.sbuf_tensor("rowq_parts", [128, 3], f32) as rowq_parts,
        nc.sbuf_tensor("rowq", [128, 1], f32) as rowq,
        nc.sbuf_tensor("srelu", [1, E_COLS], f32) as srelu,
        nc.sbuf_tensor("total", [1, 1], f32) as total,
        nc.psum_tensor("ps", [1, E_COLS], f32) as ps,
        nc.semaphore("s1") as s1,  # sync ring: m/tr/E load (+out store later)
        nc.semaphore("s2") as s2,  # sync ring: is1/em1 load
        nc.semaphore("a1") as a1,  # scalar ring: is0/em0 load
        nc.semaphore("a2") as a2,  # scalar ring: is2/em2 load
        nc.semaphore("v_sem") as v_sem,
        nc.semaphore("pe_sem") as pe_sem,
        nc.Block() as block,
    ):
        # packed layout: m | tr | E | is0 em0 | is1 em1 | is2 em2
        m_ap = xt[:, 0:BLK]
        tr_ap = xt[:, BLK : 2 * BLK]
        e_ap = xt[:, 2 * BLK : 2 * BLK + E_COLS]
        o = 2 * BLK + E_COLS  # 544

        def pair(i):  # (is_i, em_i)
            a = o + 2 * BLK * i
            return xt[:, a : a + BLK], xt[:, a + BLK : a + 2 * BLK]

        @block.sync
        def _(sync):
            sync.dma_start(out=xt[:, 0:o], in_=x[:, 0:o]).then_inc(s1, 16)
            sync.dma_start(
                out=xt[:, o + 2 * BLK : o + 4 * BLK],
                in_=x[:, o + 2 * BLK : o + 4 * BLK],
            ).then_inc(s2, 16)
            sync.wait_ge(v_sem, 9)
            sync.dma_start(out=out_d[:], in_=total[:]).then_inc(s1, 16)
            sync.wait_ge(s1, 32)

        @block.scalar
        def _(scalar):
            scalar.dma_start(
                out=xt[:, o : o + 2 * BLK], in_=x[:, o : o + 2 * BLK]
            ).then_inc(a1, 16)
            scalar.dma_start(
                out=xt[:, o + 4 * BLK :], in_=x[:, o + 4 * BLK :]
            ).then_inc(a2, 16)

        @block.vector
        def _(vector):
            is0, em0 = pair(0)
            is1, em1 = pair(1)
            is2, em2 = pair(2)
            # w = m + 1.5 * tr_m   (0.4 is deferred: relu(0.4 x) = 0.4 relu(x))
            vector.wait_ge(s1, 16)
            nc.vector.scalar_tensor_tensor(
                out=w[:], in0=tr_ap, scalar=1.5, in1=m_ap,
                op0=Alu.mult, op1=Alu.add,
            ).then_inc(v_sem, 1)
            # per chunk i: diff_i = is_i - em_i, then one fused op
            # prod_i = diff_i * w with accum_out = per-partition sum
            for i, (is_i, em_i, dma_wait) in enumerate(
                [(is0, em0, (a1, 16)), (is1, em1, (s2, 16)), (is2, em2, (a2, 16))]
            ):
                vector.wait_ge(*dma_wait)
                nc.vector.tensor_sub(diff[:, i, :], is_i, em_i).then_inc(v_sem, 1)
                vector.wait_ge(v_sem, 2 * i + 2)
                nc.vector.scalar_tensor_tensor(
                    out=prod[:, i, :], in0=diff[:, i, :], scalar=1.0,
                    in1=w[:], op0=Alu.mult, op1=Alu.mult,
                    accum_out=rowq_parts[:, i : i + 1],
                ).then_inc(v_sem, 1)
            vector.wait_ge(v_sem, 7)
            # rowq[p] = sum of the 3 chunk dots = quarter-row dot product
            nc.vector.tensor_reduce(
                rowq[:], rowq_parts[:], axis=mybir.AxisListType.X, op=Alu.add
            ).then_inc(v_sem, 1)
            vector.wait_ge(pe_sem, 1)
            # relu the 32 per-row sums, accumulate them to one scalar
            nc.vector.tensor_scalar(
                out=srelu[:], in0=ps[:], scalar1=0.0, scalar2=None,
                op0=Alu.max, op1=Alu.add, accum_out=total[:],
            ).then_inc(v_sem, 1)

        @block.tensor
        def _(tensor):
            tensor.wait_ge(s1, 16)
            tensor.wait_ge(v_sem, 8)
            # s[1, 32] = rowq^T @ E — sums each row's 4 partition-quarters
            nc.tensor.matmul(
                ps[:], rowq[:], e_ap, start=True, stop=True
            ).then_inc(pe_sem, 1)

    return nc


def pack_inputs(A_is_t, A_is_t_14, A_is_t_28, A_em_t, A_em_t_14, A_em_t_28, m, tr_m):
    idx = np.arange(B)

    def diag(a):
        return np.asarray(a)[idx, idx]  # [B, D] gather of the used diagonal

    # layout: m | tr | E | is0 em0 | is1 em1 | is2 em2
    blocks = [
        (0, np.asarray(m, dtype=np.float32)),
        (BLK, np.asarray(tr_m, dtype=np.float32)),
        (2 * BLK + E_COLS, diag(A_is_t)),
        (3 * BLK + E_COLS, diag(A_em_t)),
        (4 * BLK + E_COLS, diag(A_is_t_14)),
        (5 * BLK + E_COLS, diag(A_em_t_14)),
        (6 * BLK + E_COLS, diag(A_is_t_28)),
        (7 * BLK + E_COLS, diag(A_em_t_28)),
    ]
    X = np.empty((N_CORES, 128, FREE), dtype=np.float32)
    for off, arr in blocks:
        X[:, :, off : off + BLK] = arr.reshape(N_CORES, 128, BLK)
    X[:, :, 2 * BLK : 2 * BLK + E_COLS] = np.repeat(
        np.eye(E_COLS, dtype=np.float32), 4, axis=0
    )
    return [{"x": np.ascontiguousarray(X[c])} for c in range(N_CORES)]


def run(in_maps, **kwargs):
    global _NC_CACHE
    if _NC_CACHE is None:
        _NC_CACHE = build_nc()
    return run_bass_kernel_spmd(
        _NC_CACHE, in_maps, core_ids=list(range(N_CORES)), **kwargs
    )


def kernel(**inputs) -> np.ndarray:
    res = run(pack_inputs(**inputs))
    total = 0.4 * sum(float(r["out"][0, 0]) for r in res.results)
    return np.array([total], dtype=np.float32)
